# revision 4
# baseline (speedup 1.0000x reference)
"""AttentionDecoder single-step on 8 TRN2 NeuronCores (Bass/Tile).

Sharding (batch=1, tensor parallel):
  - LSTM gate weights column-sharded: core k owns hidden positions
    [k*256, (k+1)*256) of each of the 4 gates -> [1024, 2048] slices.
  - out_W vocab-sharded (padded 32000->32768): core k owns rows
    [k*4096, (k+1)*4096).  Per-core log-softmax stats are AllGathered.
  - comb_W row-sharded (256 rows/core); x, h1, h2 AllGathered (8KB each).
  - attention computed redundantly on every core (tiny).
  - embedding table replicated; row gathered on-device via indirect DMA.

All GEMV matmuls are weight-stationary ([128,128] bf16 tiles -> FWL), with
the activation vector as a [128,1] rhs column, so outputs land across
partitions and elementwise/reduction work is lane-parallel.
Weights are pre-transposed/pre-cast to bf16 on the host; accumulation in
fp32 PSUM.  f32 elementwise + softmax.
"""
import os
import sys

sys.path.insert(0, "/opt/trn_rl_repo")

import numpy as np
import ml_dtypes

import concourse.mybir as mybir
import concourse.bacc as bacc
import concourse.tile as tile
from concourse import bass
from concourse.bass_utils import run_bass_kernel_spmd
from concourse.masks import make_identity

BF16 = mybir.dt.bfloat16
F32 = mybir.dt.float32
I32 = mybir.dt.int32
np_bf16 = ml_dtypes.bfloat16

NC = 8
V, VP = 32000, 32768
E = H = 2048
NL = 2
A = 50
HS = H // NC          # 256 hidden positions per core
VS = VP // NC         # 4096 vocab rows per core
KT = H // 128         # 16 k-tiles over H
PAD_BIAS = -1e9       # bias for padded vocab rows

_CACHE = {}
LAST_EXEC_NS = None


def _build():
    nc = bacc.Bacc("TRN2", target_bir_lowering=False, debug=False, num_devices=NC)

    # ---- parameters ----
    idx16_d = nc.dram_tensor("idx16", [16, 1], I32, kind="ExternalInput")
    table_d = nc.dram_tensor("table", [V * 16, 128], BF16, kind="ExternalInput")
    hid0_d = nc.dram_tensor("hid0", [KT, 128], F32, kind="ExternalInput")
    hid1_d = nc.dram_tensor("hid1", [KT, 128], F32, kind="ExternalInput")
    cell0_d = nc.dram_tensor("cell0", [2, 128], F32, kind="ExternalInput")
    cell1_d = nc.dram_tensor("cell1", [2, 128], F32, kind="ExternalInput")
    enc_d = nc.dram_tensor("enc", [A, H], BF16, kind="ExternalInput")
    attnWT_d = nc.dram_tensor("attnWT", [128, 32 * A], BF16, kind="ExternalInput")
    attnb_d = nc.dram_tensor("attnb", [A, 1], F32, kind="ExternalInput")
    combWT_d = nc.dram_tensor("combWT", [128, 32 * 256], BF16, kind="ExternalInput")
    combb_d = nc.dram_tensor("combb", [128, 2], F32, kind="ExternalInput")
    wih1_d = nc.dram_tensor("wih1", [128, KT * 1024], BF16, kind="ExternalInput")
    whh1_d = nc.dram_tensor("whh1", [128, KT * 1024], BF16, kind="ExternalInput")
    wih2_d = nc.dram_tensor("wih2", [128, KT * 1024], BF16, kind="ExternalInput")
    whh2_d = nc.dram_tensor("whh2", [128, KT * 1024], BF16, kind="ExternalInput")
    bk1_d = nc.dram_tensor("bk1", [128, 8], F32, kind="ExternalInput")
    bk2_d = nc.dram_tensor("bk2", [128, 8], F32, kind="ExternalInput")
    ow_d = [
        nc.dram_tensor(f"ow{c}", [128, KT * 1024], BF16, kind="ExternalInput")
        for c in range(4)
    ]
    outbk_d = nc.dram_tensor("outbk", [128, 32], F32, kind="ExternalInput")

    lsm_d = nc.dram_tensor("lsm", [128, 32], F32, kind="ExternalOutput")
    hout_d = nc.dram_tensor("hout", [NL, 2, 128], F32, kind="ExternalOutput")
    cout_d = nc.dram_tensor("cout", [NL, 2, 128], F32, kind="ExternalOutput")
    attnw_d = nc.dram_tensor("attnw", [A, 1], F32, kind="ExternalOutput")

    RG = [list(range(NC))]

    with tile.TileContext(nc) as tc:
        with (
            tc.tile_pool(name="wpool", bufs=5) as wpool,
            tc.tile_pool(name="cpool", bufs=1) as cpool,
            tc.tile_pool(name="cst", bufs=1) as cst,
            tc.tile_pool(name="act", bufs=1) as act,
            tc.tile_pool(name="pst", bufs=1, space="PSUM") as pst,
            tc.tile_pool(name="psb", bufs=1, space="PSUM") as psb,
            tc.tile_pool(name="psmm", bufs=1, space="PSUM") as psmm,
            tc.tile_pool(name="dram", bufs=1, space="DRAM") as dram,
        ):
            # ================= weight streams (SP HWDGE ring, FIFO) ========
            combs = cpool.tile([128, 32 * 256], BF16, tag="comb")
            nc.sync.dma_start(out=combs[:, :], in_=combWT_d[:, :])
            wih1s = wpool.tile([128, KT * 1024], BF16, tag="w")
            nc.sync.dma_start(out=wih1s[:, :], in_=wih1_d[:, :])
            whh1s = wpool.tile([128, KT * 1024], BF16, tag="w")
            nc.sync.dma_start(out=whh1s[:, :], in_=whh1_d[:, :])
            wih2s = wpool.tile([128, KT * 1024], BF16, tag="w")
            nc.sync.dma_start(out=wih2s[:, :], in_=wih2_d[:, :])
            whh2s = wpool.tile([128, KT * 1024], BF16, tag="w")
            nc.sync.dma_start(out=whh2s[:, :], in_=whh2_d[:, :])
            ows = []
            for c in range(4):
                t = wpool.tile([128, KT * 1024], BF16, tag="w")
                nc.sync.dma_start(out=t[:, :], in_=ow_d[c][:, :])
                ows.append(t)

            # ================= small inputs (ACT HWDGE ring) ===============
            idx_t = cst.tile([16, 1], I32, tag="idx")
            nc.scalar.dma_start(out=idx_t[:, :], in_=idx16_d[:, :])
            enc_s = cst.tile([A, H], BF16, tag="enc")
            nc.scalar.dma_start(out=enc_s[:, :], in_=enc_d[:, :])
            attnWT_s = cst.tile([128, 32 * A], BF16, tag="attnWT")
            nc.scalar.dma_start(out=attnWT_s[:, :], in_=attnWT_d[:, :])
            attnb_s = cst.tile([A, 1], F32, tag="attnb")
            nc.scalar.dma_start(out=attnb_s[:, :], in_=attnb_d[:, :])
            combb_s = cst.tile([128, 2], F32, tag="combb")
            nc.scalar.dma_start(out=combb_s[:, :], in_=combb_d[:, :])
            bk1_s = cst.tile([128, 8], F32, tag="bk1")
            nc.scalar.dma_start(out=bk1_s[:, :], in_=bk1_d[:, :])
            bk2_s = cst.tile([128, 8], F32, tag="bk2")
            nc.scalar.dma_start(out=bk2_s[:, :], in_=bk2_d[:, :])
            outbk_s = cst.tile([128, 32], F32, tag="outbk")
            nc.scalar.dma_start(out=outbk_s[:, :], in_=outbk_d[:, :])
            ct = []
            for l, cd in enumerate((cell0_d, cell1_d)):
                t = cst.tile([128, 2], F32, tag=f"cell{l}")
                for m in range(2):
                    nc.scalar.dma_start(out=t[:, m : m + 1], in_=cd[m, :])
                ct.append(t)

            # identities for PE transposes
            idf = cst.tile([16, 16], F32, tag="idf")
            make_identity(nc, idf[:, :])
            idb = cst.tile([16, 16], BF16, tag="idb")
            nc.vector.tensor_copy(out=idb[:, :], in_=idf[:, :])

            # ================= embedding row gather =======================
            emb_st = act.tile([16, 128], BF16, tag="emb_st")
            nc.gpsimd.indirect_dma_start(
                out=emb_st[:, :],
                out_offset=None,
                in_=table_d[:, :],
                in_offset=bass.IndirectOffsetOnAxis(ap=idx_t[:, :1], axis=0),
            )
            emb_ps = pst.tile([128, 16], BF16, tag="pstb")
            nc.tensor.transpose(emb_ps[:, :], emb_st[:, :], idb[:, :])
            embc = act.tile([128, 16], BF16, tag="embc")
            nc.vector.tensor_copy(out=embc[:, :], in_=emb_ps[:, :])

            # ================= hidden -> column layout ====================
            hcols = []
            for l, hd in enumerate((hid0_d, hid1_d)):
                st = act.tile([16, 128], F32, tag=f"hst{l}")
                nc.scalar.dma_start(out=st[:, :], in_=hd[:, :])
                ps = pst.tile([128, 16], F32, tag="pstf")
                nc.tensor.transpose(ps[:, :], st[:, :], idf[:, :])
                hc = act.tile([128, 16], BF16, tag=f"hcols{l}")
                nc.vector.tensor_copy(out=hc[:, :], in_=ps[:, :])
                hcols.append(hc)

            # ================= attention ==================================
            # attn logits^T: [A,1] = sum_c attnWT[:, c*A:(c+1)*A].T @ cat_c
            at_ps = psb.tile([A, 1], F32, tag="attn")
            for c in range(32):
                rhs = embc[:, c : c + 1] if c < 16 else hcols[0][:, c - 16 : c - 15]
                nc.tensor.matmul(
                    at_ps[:, :],
                    attnWT_s[:, c * A : (c + 1) * A],
                    rhs,
                    start=(c == 0),
                    stop=(c == 31),
                )
            at_sb = act.tile([A, 1], F32, tag="at_sb")
            nc.vector.tensor_tensor(
                out=at_sb[:, :], in0=at_ps[:, :], in1=attnb_s[:, :],
                op=mybir.AluOpType.add,
            )
            # softmax over the 50 partitions
            amax = act.tile([A, 1], F32, tag="amax")
            nc.gpsimd.partition_all_reduce(
                amax[:, :], at_sb[:, :], A, bass.bass_isa.ReduceOp.max
            )
            namax = act.tile([A, 1], F32, tag="namax")
            nc.vector.tensor_scalar_mul(namax[:, :], amax[:, :], -1.0)
            aexp = act.tile([A, 1], F32, tag="aexp")
            nc.scalar.activation(
                aexp[:, :], at_sb[:, :], mybir.ActivationFunctionType.Exp,
                bias=namax[:, :1],
            )
            asum = act.tile([A, 1], F32, tag="asum")
            nc.gpsimd.partition_all_reduce(
                asum[:, :], aexp[:, :], A, bass.bass_isa.ReduceOp.add
            )
            arec = act.tile([A, 1], F32, tag="arec")
            nc.vector.reciprocal(arec[:, :], asum[:, :])
            awT = act.tile([A, 1], F32, tag="awT")
            nc.vector.tensor_scalar_mul(awT[:, :], aexp[:, :], arec[:, :1])
            nc.scalar.dma_start(out=attnw_d[:, :], in_=awT[:, :])
            awTb = act.tile([A, 1], BF16, tag="awTb")
            nc.vector.tensor_copy(out=awTb[:, :], in_=awT[:, :])

            # attn_applied^T as columns: [128,16]
            aa_ps = psb.tile([128, 16], F32, tag="aa")
            for c in range(16):
                nc.tensor.matmul(
                    aa_ps[:, c : c + 1],
                    enc_s[:, c * 128 : (c + 1) * 128],
                    awTb[:, :],
                    start=True,
                    stop=True,
                )
            aac = act.tile([128, 16], BF16, tag="aac")
            nc.vector.tensor_copy(out=aac[:, :], in_=aa_ps[:, :])

            # ================= comb: x_k = combW_k @ [emb; attn] + b ======
            x_ps = psb.tile([128, 2], F32, tag="x")
            for m in range(2):
                for c in range(32):
                    rhs = embc[:, c : c + 1] if c < 16 else aac[:, c - 16 : c - 15]
                    nc.tensor.matmul(
                        x_ps[:, m : m + 1],
                        combs[:, c * 256 + m * 128 : c * 256 + (m + 1) * 128],
                        rhs,
                        start=(c == 0),
                        stop=(c == 31),
                    )
            x_sb = act.tile([128, 2], F32, tag="x_sb")
            nc.vector.tensor_tensor(
                out=x_sb[:, :], in0=x_ps[:, :], in1=combb_s[:, :],
                op=mybir.AluOpType.add,
            )

            # ---- AllGather x ----
            agx_i = dram.tile([1, HS], F32, tag="agx_i")
            agx_o = dram.tile([KT, 128], F32, tag="agx_o")
            for m in range(2):
                nc.scalar.dma_start(
                    out=agx_i[0, m * 128 : (m + 1) * 128], in_=x_sb[:, m : m + 1]
                )
            nc.gpsimd.collective_compute(
                "AllGather", mybir.AluOpType.bypass, replica_groups=RG,
                ins=[agx_i.opt()], outs=[agx_o.opt()],
            )
            xg_st = act.tile([KT, 128], F32, tag="xg_st")
            nc.scalar.dma_start(out=xg_st[:, :], in_=agx_o[:, :])
            xg_ps = pst.tile([128, KT], F32, tag="pstf")
            nc.tensor.transpose(xg_ps[:, :], xg_st[:, :], idf[:, :])
            xc = act.tile([128, KT], BF16, tag="xc")
            nc.vector.tensor_copy(out=xc[:, :], in_=xg_ps[:, :])

            # ================= LSTM layers ================================
            def lstm_layer(l, xin, wih_s, whh_s, bk_s, ct_l, agh_i, agh_o):
                g_ps = psmm.tile([128, 8], F32, tag="g")
                for m in range(8):
                    for k in range(KT):
                        nc.tensor.matmul(
                            g_ps[:, m : m + 1],
                            wih_s[:, k * 1024 + m * 128 : k * 1024 + (m + 1) * 128],
                            xin[:, k : k + 1],
                            start=(k == 0),
                            stop=False,
                        )
                    for k in range(KT):
                        nc.tensor.matmul(
                            g_ps[:, m : m + 1],
                            whh_s[:, k * 1024 + m * 128 : k * 1024 + (m + 1) * 128],
                            hcols[l][:, k : k + 1],
                            start=False,
                            stop=(k == KT - 1),
                        )
                g_sb = act.tile([128, 8], F32, tag=f"g_sb{l}")
                nc.vector.tensor_tensor(
                    out=g_sb[:, :], in0=g_ps[:, :], in1=bk_s[:, :],
                    op=mybir.AluOpType.add,
                )
                sif = act.tile([128, 4], F32, tag=f"sif{l}")
                nc.scalar.activation(
                    sif[:, :], g_sb[:, 0:4], mybir.ActivationFunctionType.Sigmoid
                )
                tg = act.tile([128, 2], F32, tag=f"tg{l}")
                nc.scalar.activation(
                    tg[:, :], g_sb[:, 4:6], mybir.ActivationFunctionType.Tanh
                )
                so = act.tile([128, 2], F32, tag=f"so{l}")
                nc.scalar.activation(
                    so[:, :], g_sb[:, 6:8], mybir.ActivationFunctionType.Sigmoid
                )
                t1 = act.tile([128, 2], F32, tag=f"t1{l}")
                nc.vector.tensor_tensor(
                    out=t1[:, :], in0=sif[:, 2:4], in1=ct_l[:, :],
                    op=mybir.AluOpType.mult,
                )
                t2 = act.tile([128, 2], F32, tag=f"t2{l}")
                nc.vector.tensor_tensor(
                    out=t2[:, :], in0=sif[:, 0:2], in1=tg[:, :],
                    op=mybir.AluOpType.mult,
                )
                cnew = act.tile([128, 2], F32, tag=f"cnew{l}")
                nc.vector.tensor_tensor(
                    out=cnew[:, :], in0=t1[:, :], in1=t2[:, :],
                    op=mybir.AluOpType.add,
                )
                tcn = act.tile([128, 2], F32, tag=f"tcn{l}")
                nc.scalar.activation(
                    tcn[:, :], cnew[:, :], mybir.ActivationFunctionType.Tanh
                )
                hnew = act.tile([128, 2], F32, tag=f"hnew{l}")
                nc.vector.tensor_tensor(
                    out=hnew[:, :], in0=so[:, :], in1=tcn[:, :],
                    op=mybir.AluOpType.mult,
                )
                for m in range(2):
                    nc.scalar.dma_start(
                        out=cout_d[l, m, :], in_=cnew[:, m : m + 1]
                    )
                    nc.scalar.dma_start(
                        out=hout_d[l, m, :], in_=hnew[:, m : m + 1]
                    )
                    nc.scalar.dma_start(
                        out=agh_i[0, m * 128 : (m + 1) * 128],
                        in_=hnew[:, m : m + 1],
                    )
                nc.gpsimd.collective_compute(
                    "AllGather", mybir.AluOpType.bypass, replica_groups=RG,
                    ins=[agh_i.opt()], outs=[agh_o.opt()],
                )
                hg_st = act.tile([KT, 128], F32, tag=f"hg_st{l}")
                nc.scalar.dma_start(out=hg_st[:, :], in_=agh_o[:, :])
                hg_ps = pst.tile([128, KT], F32, tag="pstf")
                nc.tensor.transpose(hg_ps[:, :], hg_st[:, :], idf[:, :])
                hgc = act.tile([128, KT], BF16, tag=f"hgc{l}")
                nc.vector.tensor_copy(out=hgc[:, :], in_=hg_ps[:, :])
                return hgc

            agh1_i = dram.tile([1, HS], F32, tag="agh1_i")
            agh1_o = dram.tile([KT, 128], F32, tag="agh1_o")
            h1c = lstm_layer(0, xc, wih1s, whh1s, bk1_s, ct[0], agh1_i, agh1_o)
            agh2_i = dram.tile([1, HS], F32, tag="agh2_i")
            agh2_o = dram.tile([KT, 128], F32, tag="agh2_o")
            h2c = lstm_layer(1, h1c, wih2s, whh2s, bk2_s, ct[1], agh2_i, agh2_o)

            # ================= output projection ==========================
            l_ps = psmm.tile([128, 32], F32, tag="l")
            for c in range(4):
                for mm in range(8):
                    m = c * 8 + mm
                    for k in range(KT):
                        nc.tensor.matmul(
                            l_ps[:, m : m + 1],
                            ows[c][:, k * 1024 + mm * 128 : k * 1024 + (mm + 1) * 128],
                            h2c[:, k : k + 1],
                            start=(k == 0),
                            stop=(k == KT - 1),
                        )
            l_sb = act.tile([128, 32], F32, tag="l_sb")
            nc.vector.tensor_tensor(
                out=l_sb[:, :], in0=l_ps[:, :], in1=outbk_s[:, :],
                op=mybir.AluOpType.add,
            )

            # ---- local log-softmax stats ----
            mx = act.tile([128, 1], F32, tag="mx")
            nc.vector.tensor_reduce(
                out=mx[:, :], in_=l_sb[:, :], axis=mybir.AxisListType.X,
                op=mybir.AluOpType.max,
            )
            mxb = act.tile([128, 1], F32, tag="mxb")
            nc.gpsimd.partition_all_reduce(
                mxb[:, :], mx[:, :], 128, bass.bass_isa.ReduceOp.max
            )
            nmx = act.tile([128, 1], F32, tag="nmx")
            nc.vector.tensor_scalar_mul(nmx[:, :], mxb[:, :], -1.0)
            ex = act.tile([128, 32], F32, tag="ex")
            sx = act.tile([128, 1], F32, tag="sx")
            nc.scalar.activation(
                ex[:, :], l_sb[:, :], mybir.ActivationFunctionType.Exp,
                bias=nmx[:, :1], accum_out=sx[:, :],
            )
            sxb = act.tile([128, 1], F32, tag="sxb")
            nc.gpsimd.partition_all_reduce(
                sxb[:, :], sx[:, :], 128, bass.bass_isa.ReduceOp.add
            )

            # ---- AllGather (M_k, S_k) ----
            agst_i = dram.tile([1, 8], F32, tag="agst_i")
            agst_o = dram.tile([8, 8], F32, tag="agst_o")
            stt = act.tile([1, 8], F32, tag="stt")
            nc.vector.memset(stt[:, :], 0.0)
            nc.vector.tensor_copy(out=stt[:, 0:1], in_=mxb[0:1, 0:1])
            nc.vector.tensor_copy(out=stt[:, 1:2], in_=sxb[0:1, 0:1])
            nc.scalar.dma_start(out=agst_i[:, :], in_=stt[:, :])
            nc.gpsimd.collective_compute(
                "AllGather", mybir.AluOpType.bypass, replica_groups=RG,
                ins=[agst_i.opt()], outs=[agst_o.opt()],
            )
            st8 = act.tile([1, 8, 8], F32, tag="st8")
            nc.scalar.dma_start(out=st8[:, :, :], in_=agst_o[:, :])
            mg = act.tile([1, 1], F32, tag="mg")
            nc.vector.tensor_reduce(
                out=mg[:, :], in_=st8[:, :, 0], axis=mybir.AxisListType.X,
                op=mybir.AluOpType.max,
            )
            nmg = act.tile([1, 1], F32, tag="nmg")
            nc.vector.tensor_scalar_mul(nmg[:, :], mg[:, :], -1.0)
            e8 = act.tile([1, 8], F32, tag="e8")
            nc.scalar.activation(
                e8[:, :], st8[:, :, 0], mybir.ActivationFunctionType.Exp,
                bias=nmg[:, :1],
            )
            t8 = act.tile([1, 8], F32, tag="t8")
            nc.vector.tensor_tensor(
                out=t8[:, :], in0=e8[:, :], in1=st8[:, :, 1],
                op=mybir.AluOpType.mult,
            )
            zg = act.tile([1, 1], F32, tag="zg")
            nc.vector.tensor_reduce(
                out=zg[:, :], in_=t8[:, :], axis=mybir.AxisListType.X,
                op=mybir.AluOpType.add,
            )
            lz = act.tile([1, 1], F32, tag="lz")
            nc.scalar.activation(
                lz[:, :], zg[:, :], mybir.ActivationFunctionType.Ln
            )
            shf = act.tile([1, 1], F32, tag="shf")
            nc.vector.tensor_tensor(
                out=shf[:, :], in0=lz[:, :], in1=mg[:, :], op=mybir.AluOpType.add,
            )
            nshf = act.tile([1, 1], F32, tag="nshf")
            nc.vector.tensor_scalar_mul(nshf[:, :], shf[:, :], -1.0)
            nshb = act.tile([128, 1], F32, tag="nshb")
            nc.gpsimd.partition_broadcast(nshb[:, :], nshf[:, :])
            lsm_sb = act.tile([128, 32], F32, tag="lsm_sb")
            nc.scalar.activation(
                lsm_sb[:, :], l_sb[:, :], mybir.ActivationFunctionType.Identity,
                bias=nshb[:, :1],
            )
            nc.scalar.dma_start(out=lsm_d[:, :], in_=lsm_sb[:, :])

    nc.compile()
    return nc


def _get_nc():
    if "nc" not in _CACHE:
        _CACHE["nc"] = _build()
    return _CACHE["nc"]


def _prep_in_maps(inputs):
    inp = np.asarray(inputs["input"]).reshape(-1)
    hidden = np.asarray(inputs["hidden"], np.float32)
    cell = np.asarray(inputs["cell"], np.float32)
    enc = np.asarray(inputs["encoder_outputs"], np.float32)
    table = np.asarray(inputs["embed_table"], np.float32)
    attn_W = np.asarray(inputs["attn_W"], np.float32)
    attn_b = np.asarray(inputs["attn_b"], np.float32)
    comb_W = np.asarray(inputs["comb_W"], np.float32)
    comb_b = np.asarray(inputs["comb_b"], np.float32)
    Wih = np.asarray(inputs["Wih"], np.float32)
    Whh = np.asarray(inputs["Whh"], np.float32)
    bih = np.asarray(inputs["bih"], np.float32)
    bhh = np.asarray(inputs["bhh"], np.float32)
    out_W = np.asarray(inputs["out_W"], np.float32)
    out_b = np.asarray(inputs["out_b"], np.float32)

    def kmaj(wt):  # [K, M] -> [128, (K/128)*M] k-tile-major slab
        K, M = wt.shape
        return np.ascontiguousarray(
            wt.reshape(K // 128, 128, M).transpose(1, 0, 2).reshape(128, -1)
        ).astype(np_bf16)

    idx16 = (int(inp[0]) * 16 + np.arange(16, dtype=np.int32)).reshape(16, 1)
    table_bf = np.ascontiguousarray(table.astype(np_bf16).reshape(V * 16, 128))
    hid = [np.ascontiguousarray(hidden[l, 0].reshape(KT, 128)) for l in range(NL)]
    enc_bf = np.ascontiguousarray(enc.astype(np_bf16))
    attnWT = kmaj(np.ascontiguousarray(attn_W.T))
    attnb = np.ascontiguousarray(attn_b.reshape(A, 1))

    # padded out projection
    owp = np.zeros((VP, H), np.float32)
    owp[:V] = out_W
    obp = np.full((VP,), PAD_BIAS, np.float32)
    obp[:V] = out_b

    b2 = bih + bhh  # [NL, 4H]

    common = {
        "idx16": idx16, "table": table_bf, "hid0": hid[0], "hid1": hid[1],
        "enc": enc_bf, "attnWT": attnWT, "attnb": attnb,
    }
    in_maps = []
    for k in range(NC):
        m = dict(common)
        for l in range(NL):
            m[f"cell{l}"] = np.ascontiguousarray(
                cell[l, 0, k * HS : (k + 1) * HS].reshape(2, 128)
            )
        cslice = comb_W[k * HS : (k + 1) * HS]  # [256, 4096]
        m["combWT"] = kmaj(np.ascontiguousarray(cslice.T))
        m["combb"] = np.ascontiguousarray(
            comb_b[k * HS : (k + 1) * HS].reshape(2, 128).T
        )
        for l in range(NL):
            rows = np.concatenate(
                [Wih[l][g * H + k * HS : g * H + (k + 1) * HS] for g in range(4)]
            )  # [1024, 2048]
            m[f"wih{l + 1}"] = kmaj(np.ascontiguousarray(rows.T))
            rows = np.concatenate(
                [Whh[l][g * H + k * HS : g * H + (k + 1) * HS] for g in range(4)]
            )
            m[f"whh{l + 1}"] = kmaj(np.ascontiguousarray(rows.T))
            bk = np.concatenate(
                [b2[l][g * H + k * HS : g * H + (k + 1) * HS] for g in range(4)]
            )  # [1024]
            m[f"bk{l + 1}"] = np.ascontiguousarray(bk.reshape(8, 128).T)
        oslice = np.ascontiguousarray(owp[k * VS : (k + 1) * VS].T)  # [2048, 4096]
        for c in range(4):
            m[f"ow{c}"] = kmaj(np.ascontiguousarray(oslice[:, c * 1024 : (c + 1) * 1024]))
        m["outbk"] = np.ascontiguousarray(
            obp[k * VS : (k + 1) * VS].reshape(32, 128).T
        )
        in_maps.append(m)
    return in_maps


def kernel(**inputs):
    global LAST_EXEC_NS
    import time as _time

    _t0 = _time.time()
    nc = _get_nc()
    print(f"[kernel] graph ready {_time.time() - _t0:.1f}s", file=sys.stderr, flush=True)
    in_maps = _prep_in_maps(inputs)
    print(f"[kernel] host prep done {_time.time() - _t0:.1f}s", file=sys.stderr, flush=True)
    trace = bool(int(os.environ.get("KERNEL_TRACE", "0")))
    res = run_bass_kernel_spmd(
        nc, in_maps, core_ids=list(range(NC)), trace=trace
    )
    print(f"[kernel] run done {_time.time() - _t0:.1f}s", file=sys.stderr, flush=True)
    LAST_EXEC_NS = res.exec_time_ns

    outs = res.results
    lsm = np.concatenate(
        [np.asarray(outs[k]["lsm"]).T.reshape(-1) for k in range(NC)]
    )[:V].reshape(1, V).astype(np.float32)
    h_new = np.stack(
        [
            np.concatenate(
                [np.asarray(outs[k]["hout"][l]).reshape(-1) for k in range(NC)]
            ).reshape(1, H)
            for l in range(NL)
        ]
    )
    c_new = np.stack(
        [
            np.concatenate(
                [np.asarray(outs[k]["cout"][l]).reshape(-1) for k in range(NC)]
            ).reshape(1, H)
            for l in range(NL)
        ]
    )
    attnw = np.asarray(outs[0]["attnw"]).reshape(1, A).astype(np.float32)
    return lsm, h_new, c_new, attnw


# revision 7
# speedup vs baseline: 1.0905x; 1.0905x over previous
"""AttentionDecoder single-step on 8 TRN2 NeuronCores (Bass/Tile).

Sharding (batch=1, tensor parallel):
  - LSTM gate weights column-sharded: core k owns hidden positions
    [k*256, (k+1)*256) of each of the 4 gates -> [1024, 2048] slices.
  - out_W vocab-sharded (padded 32000->32768): core k owns rows
    [k*4096, (k+1)*4096).  Per-core log-softmax stats are AllGathered.
  - comb_W row-sharded (256 rows/core); x, h1, h2 AllGathered (8KB each).
  - attention computed redundantly on every core (tiny).
  - embedding table replicated; row gathered on-device via indirect DMA.

All GEMV matmuls are weight-stationary ([128,128] bf16 tiles -> FWL), with
the activation vector as a [128,1] rhs column, so outputs land across
partitions and elementwise/reduction work is lane-parallel.
Weights are pre-transposed/pre-cast to bf16 on the host; accumulation in
fp32 PSUM.  f32 elementwise + softmax.
"""
import os
import sys

sys.path.insert(0, "/opt/trn_rl_repo")

import numpy as np
import ml_dtypes

import concourse.mybir as mybir
import concourse.bacc as bacc
import concourse.tile as tile
from concourse import bass
from concourse.bass_utils import run_bass_kernel_spmd
from concourse.masks import make_identity

BF16 = mybir.dt.bfloat16
F32 = mybir.dt.float32
I32 = mybir.dt.int32
np_bf16 = ml_dtypes.bfloat16

NC = 8
V, VP = 32000, 32768
E = H = 2048
NL = 2
A = 50
HS = H // NC          # 256 hidden positions per core
VS = VP // NC         # 4096 vocab rows per core
KT = H // 128         # 16 k-tiles over H
PAD_BIAS = -1e9       # bias for padded vocab rows

_CACHE = {}
LAST_EXEC_NS = None


def _build():
    nc = bacc.Bacc("TRN2", target_bir_lowering=False, debug=False, num_devices=NC)

    # ---- parameters ----
    idx16_d = nc.dram_tensor("idx16", [16, 1], I32, kind="ExternalInput")
    table_d = nc.dram_tensor("table", [V * 16, 128], BF16, kind="ExternalInput")
    hid0_d = nc.dram_tensor("hid0", [KT, 128], F32, kind="ExternalInput")
    hid1_d = nc.dram_tensor("hid1", [KT, 128], F32, kind="ExternalInput")
    cell0_d = nc.dram_tensor("cell0", [2, 128], F32, kind="ExternalInput")
    cell1_d = nc.dram_tensor("cell1", [2, 128], F32, kind="ExternalInput")
    enc_d = nc.dram_tensor("enc", [A, H], BF16, kind="ExternalInput")
    attnWT_d = nc.dram_tensor("attnWT", [128, 32 * A], BF16, kind="ExternalInput")
    attnb_d = nc.dram_tensor("attnb", [A, 1], F32, kind="ExternalInput")
    combWT_d = nc.dram_tensor("combWT", [128, 32 * 256], BF16, kind="ExternalInput")
    combb_d = nc.dram_tensor("combb", [128, 2], F32, kind="ExternalInput")
    wih1_d = nc.dram_tensor("wih1", [128, KT * 1024], BF16, kind="ExternalInput")
    whh1_d = nc.dram_tensor("whh1", [128, KT * 1024], BF16, kind="ExternalInput")
    wih2_d = nc.dram_tensor("wih2", [128, KT * 1024], BF16, kind="ExternalInput")
    whh2_d = nc.dram_tensor("whh2", [128, KT * 1024], BF16, kind="ExternalInput")
    bk1_d = nc.dram_tensor("bk1", [128, 8], F32, kind="ExternalInput")
    bk2_d = nc.dram_tensor("bk2", [128, 8], F32, kind="ExternalInput")
    ow_d = [
        nc.dram_tensor(f"ow{c}", [128, KT * 1024], BF16, kind="ExternalInput")
        for c in range(4)
    ]
    outbk_d = nc.dram_tensor("outbk", [128, 32], F32, kind="ExternalInput")

    lsm_d = nc.dram_tensor("lsm", [128, 32], F32, kind="ExternalOutput")
    hout_d = nc.dram_tensor("hout", [NL, 2, 128], F32, kind="ExternalOutput")
    cout_d = nc.dram_tensor("cout", [NL, 2, 128], F32, kind="ExternalOutput")
    attnw_d = nc.dram_tensor("attnw", [A, 1], F32, kind="ExternalOutput")

    RG = [list(range(NC))]

    with tile.TileContext(nc) as tc:
        with (
            tc.tile_pool(name="wpool", bufs=5) as wpool,
            tc.tile_pool(name="cpool", bufs=1) as cpool,
            tc.tile_pool(name="cst", bufs=1) as cst,
            tc.tile_pool(name="act", bufs=1) as act,
            tc.tile_pool(name="pst", bufs=1, space="PSUM") as pst,
            tc.tile_pool(name="psb", bufs=1, space="PSUM") as psb,
            tc.tile_pool(name="psmm", bufs=1, space="PSUM") as psmm,
            tc.tile_pool(name="dram", bufs=1, space="DRAM") as dram,
        ):
            # ================= weight streams (SP HWDGE ring, FIFO) ========
            # ~1MB chunks: keeps per-engine queue drains short so small/bounce
            # DMAs on the other rings interleave with sub-us latency.
            def stream(dst, src, nchunks):
                total = dst.shape[1]
                step = total // nchunks
                for c in range(nchunks):
                    sl = slice(c * step, (c + 1) * step)
                    nc.sync.dma_start(out=dst[:, sl], in_=src[:, sl])

            combs = cpool.tile([128, 32 * 256], BF16, tag="comb")
            stream(combs, combWT_d, 2)
            wih1s = wpool.tile([128, KT * 1024], BF16, tag="w")
            stream(wih1s, wih1_d, 4)
            whh1s = wpool.tile([128, KT * 1024], BF16, tag="w")
            stream(whh1s, whh1_d, 4)
            wih2s = wpool.tile([128, KT * 1024], BF16, tag="w")
            stream(wih2s, wih2_d, 4)
            whh2s = wpool.tile([128, KT * 1024], BF16, tag="w")
            stream(whh2s, whh2_d, 4)
            ows = []
            for c in range(4):
                t = wpool.tile([128, KT * 1024], BF16, tag="w")
                stream(t, ow_d[c], 4)
                ows.append(t)

            # ================= small inputs (ACT HWDGE ring) ===============
            idx_t = cst.tile([16, 1], I32, tag="idx")
            nc.scalar.dma_start(out=idx_t[:, :], in_=idx16_d[:, :])
            enc_s = cst.tile([A, H], BF16, tag="enc")
            nc.scalar.dma_start(out=enc_s[:, :], in_=enc_d[:, :])
            attnWT_s = cst.tile([128, 32 * A], BF16, tag="attnWT")
            nc.scalar.dma_start(out=attnWT_s[:, :], in_=attnWT_d[:, :])
            attnb_s = cst.tile([A, 1], F32, tag="attnb")
            nc.scalar.dma_start(out=attnb_s[:, :], in_=attnb_d[:, :])
            combb_s = cst.tile([128, 2], F32, tag="combb")
            nc.scalar.dma_start(out=combb_s[:, :], in_=combb_d[:, :])
            bk1_s = cst.tile([128, 8], F32, tag="bk1")
            nc.scalar.dma_start(out=bk1_s[:, :], in_=bk1_d[:, :])
            bk2_s = cst.tile([128, 8], F32, tag="bk2")
            nc.scalar.dma_start(out=bk2_s[:, :], in_=bk2_d[:, :])
            outbk_s = cst.tile([128, 32], F32, tag="outbk")
            nc.scalar.dma_start(out=outbk_s[:, :], in_=outbk_d[:, :])
            ct = []
            for l, cd in enumerate((cell0_d, cell1_d)):
                t = cst.tile([128, 2], F32, tag=f"cell{l}")
                for m in range(2):
                    nc.scalar.dma_start(out=t[:, m : m + 1], in_=cd[m, :])
                ct.append(t)

            # identities for PE transposes
            idf = cst.tile([16, 16], F32, tag="idf")
            make_identity(nc, idf[:, :])
            idb = cst.tile([16, 16], BF16, tag="idb")
            nc.vector.tensor_copy(out=idb[:, :], in_=idf[:, :])

            # ================= embedding row gather =======================
            emb_st = act.tile([16, 128], BF16, tag="emb_st")
            nc.gpsimd.indirect_dma_start(
                out=emb_st[:, :],
                out_offset=None,
                in_=table_d[:, :],
                in_offset=bass.IndirectOffsetOnAxis(ap=idx_t[:, :1], axis=0),
            )
            emb_ps = pst.tile([128, 16], BF16, tag="pstb")
            nc.tensor.transpose(emb_ps[:, :], emb_st[:, :], idb[:, :])
            embc = act.tile([128, 16], BF16, tag="embc")
            nc.vector.tensor_copy(out=embc[:, :], in_=emb_ps[:, :])

            # ================= hidden -> column layout ====================
            hcols = []
            for l, hd in enumerate((hid0_d, hid1_d)):
                st = act.tile([16, 128], F32, tag=f"hst{l}")
                nc.scalar.dma_start(out=st[:, :], in_=hd[:, :])
                ps = pst.tile([128, 16], F32, tag="pstf")
                nc.tensor.transpose(ps[:, :], st[:, :], idf[:, :])
                hc = act.tile([128, 16], BF16, tag=f"hcols{l}")
                nc.vector.tensor_copy(out=hc[:, :], in_=ps[:, :])
                hcols.append(hc)

            # ================= attention ==================================
            # attn logits^T: [A,1] = sum_c attnWT[:, c*A:(c+1)*A].T @ cat_c
            at_ps = psb.tile([A, 1], F32, tag="attn")
            for c in range(32):
                rhs = embc[:, c : c + 1] if c < 16 else hcols[0][:, c - 16 : c - 15]
                nc.tensor.matmul(
                    at_ps[:, :],
                    attnWT_s[:, c * A : (c + 1) * A],
                    rhs,
                    start=(c == 0),
                    stop=(c == 31),
                )
            at_sb = act.tile([A, 1], F32, tag="at_sb")
            nc.vector.tensor_tensor(
                out=at_sb[:, :], in0=at_ps[:, :], in1=attnb_s[:, :],
                op=mybir.AluOpType.add,
            )
            # softmax over the 50 partitions
            amax = act.tile([A, 1], F32, tag="amax")
            nc.gpsimd.partition_all_reduce(
                amax[:, :], at_sb[:, :], A, bass.bass_isa.ReduceOp.max
            )
            namax = act.tile([A, 1], F32, tag="namax")
            nc.vector.tensor_scalar_mul(namax[:, :], amax[:, :], -1.0)
            aexp = act.tile([A, 1], F32, tag="aexp")
            nc.scalar.activation(
                aexp[:, :], at_sb[:, :], mybir.ActivationFunctionType.Exp,
                bias=namax[:, :1],
            )
            asum = act.tile([A, 1], F32, tag="asum")
            nc.gpsimd.partition_all_reduce(
                asum[:, :], aexp[:, :], A, bass.bass_isa.ReduceOp.add
            )
            arec = act.tile([A, 1], F32, tag="arec")
            nc.vector.reciprocal(arec[:, :], asum[:, :])
            awT = act.tile([A, 1], F32, tag="awT")
            nc.vector.tensor_scalar_mul(awT[:, :], aexp[:, :], arec[:, :1])
            nc.scalar.dma_start(out=attnw_d[:, :], in_=awT[:, :])
            awTb = act.tile([A, 1], BF16, tag="awTb")
            nc.vector.tensor_copy(out=awTb[:, :], in_=awT[:, :])

            # attn_applied^T as columns: [128,16]
            aa_ps = psb.tile([128, 16], F32, tag="aa")
            for c in range(16):
                nc.tensor.matmul(
                    aa_ps[:, c : c + 1],
                    enc_s[:, c * 128 : (c + 1) * 128],
                    awTb[:, :],
                    start=True,
                    stop=True,
                )
            aac = act.tile([128, 16], BF16, tag="aac")
            nc.vector.tensor_copy(out=aac[:, :], in_=aa_ps[:, :])

            # ================= comb: x_k = combW_k @ [emb; attn] + b ======
            x_ps = psb.tile([128, 2], F32, tag="x")
            for m in range(2):
                for c in range(32):
                    rhs = embc[:, c : c + 1] if c < 16 else aac[:, c - 16 : c - 15]
                    nc.tensor.matmul(
                        x_ps[:, m : m + 1],
                        combs[:, c * 256 + m * 128 : c * 256 + (m + 1) * 128],
                        rhs,
                        start=(c == 0),
                        stop=(c == 31),
                    )
            x_sb = act.tile([128, 2], F32, tag="x_sb")
            nc.vector.tensor_tensor(
                out=x_sb[:, :], in0=x_ps[:, :], in1=combb_s[:, :],
                op=mybir.AluOpType.add,
            )

            # ---- AllGather x ----
            agx_i = dram.tile([1, HS], F32, tag="agx_i")
            agx_o = dram.tile([KT, 128], F32, tag="agx_o")
            for m in range(2):
                nc.scalar.dma_start(
                    out=agx_i[0, m * 128 : (m + 1) * 128], in_=x_sb[:, m : m + 1]
                )
            nc.gpsimd.collective_compute(
                "AllGather", mybir.AluOpType.bypass, replica_groups=RG,
                ins=[agx_i.opt()], outs=[agx_o.opt()],
            )
            xg_st = act.tile([KT, 128], F32, tag="xg_st")
            nc.scalar.dma_start(out=xg_st[:, :], in_=agx_o[:, :])
            xg_ps = pst.tile([128, KT], F32, tag="pstf")
            nc.tensor.transpose(xg_ps[:, :], xg_st[:, :], idf[:, :])
            xc = act.tile([128, KT], BF16, tag="xc")
            nc.vector.tensor_copy(out=xc[:, :], in_=xg_ps[:, :])

            # ================= LSTM layers ================================
            def lstm_layer(l, xin, wih_s, whh_s, bk_s, ct_l, agh_i, agh_o):
                g_ps = psmm.tile([128, 8], F32, tag="g")
                for m in range(8):
                    for k in range(KT):
                        nc.tensor.matmul(
                            g_ps[:, m : m + 1],
                            wih_s[:, k * 1024 + m * 128 : k * 1024 + (m + 1) * 128],
                            xin[:, k : k + 1],
                            start=(k == 0),
                            stop=False,
                        )
                    for k in range(KT):
                        nc.tensor.matmul(
                            g_ps[:, m : m + 1],
                            whh_s[:, k * 1024 + m * 128 : k * 1024 + (m + 1) * 128],
                            hcols[l][:, k : k + 1],
                            start=False,
                            stop=(k == KT - 1),
                        )
                g_sb = act.tile([128, 8], F32, tag=f"g_sb{l}")
                nc.vector.tensor_tensor(
                    out=g_sb[:, :], in0=g_ps[:, :], in1=bk_s[:, :],
                    op=mybir.AluOpType.add,
                )
                # sigmoid(x) = 0.5*tanh(0.5x)+0.5 -- keeps ACT on one LUT
                # (Tanh) for both layers: no per-gate table reloads.
                tif = act.tile([128, 4], F32, tag=f"tif{l}")
                nc.scalar.activation(
                    tif[:, :], g_sb[:, 0:4], mybir.ActivationFunctionType.Tanh,
                    scale=0.5,
                )
                to = act.tile([128, 2], F32, tag=f"to{l}")
                nc.scalar.activation(
                    to[:, :], g_sb[:, 6:8], mybir.ActivationFunctionType.Tanh,
                    scale=0.5,
                )
                tg = act.tile([128, 2], F32, tag=f"tg{l}")
                nc.scalar.activation(
                    tg[:, :], g_sb[:, 4:6], mybir.ActivationFunctionType.Tanh
                )
                sif = act.tile([128, 4], F32, tag=f"sif{l}")
                nc.vector.tensor_scalar(
                    sif[:, :], tif[:, :], 0.5, 0.5,
                    mybir.AluOpType.mult, mybir.AluOpType.add,
                )
                so = act.tile([128, 2], F32, tag=f"so{l}")
                nc.vector.tensor_scalar(
                    so[:, :], to[:, :], 0.5, 0.5,
                    mybir.AluOpType.mult, mybir.AluOpType.add,
                )
                t1 = act.tile([128, 2], F32, tag=f"t1{l}")
                nc.vector.tensor_tensor(
                    out=t1[:, :], in0=sif[:, 2:4], in1=ct_l[:, :],
                    op=mybir.AluOpType.mult,
                )
                t2 = act.tile([128, 2], F32, tag=f"t2{l}")
                nc.vector.tensor_tensor(
                    out=t2[:, :], in0=sif[:, 0:2], in1=tg[:, :],
                    op=mybir.AluOpType.mult,
                )
                cnew = act.tile([128, 2], F32, tag=f"cnew{l}")
                nc.vector.tensor_tensor(
                    out=cnew[:, :], in0=t1[:, :], in1=t2[:, :],
                    op=mybir.AluOpType.add,
                )
                tcn = act.tile([128, 2], F32, tag=f"tcn{l}")
                nc.scalar.activation(
                    tcn[:, :], cnew[:, :], mybir.ActivationFunctionType.Tanh
                )
                hnew = act.tile([128, 2], F32, tag=f"hnew{l}")
                nc.vector.tensor_tensor(
                    out=hnew[:, :], in0=so[:, :], in1=tcn[:, :],
                    op=mybir.AluOpType.mult,
                )
                for m in range(2):
                    nc.scalar.dma_start(
                        out=cout_d[l, m, :], in_=cnew[:, m : m + 1]
                    )
                    nc.scalar.dma_start(
                        out=hout_d[l, m, :], in_=hnew[:, m : m + 1]
                    )
                    nc.scalar.dma_start(
                        out=agh_i[0, m * 128 : (m + 1) * 128],
                        in_=hnew[:, m : m + 1],
                    )
                nc.gpsimd.collective_compute(
                    "AllGather", mybir.AluOpType.bypass, replica_groups=RG,
                    ins=[agh_i.opt()], outs=[agh_o.opt()],
                )
                hg_st = act.tile([KT, 128], F32, tag=f"hg_st{l}")
                nc.scalar.dma_start(out=hg_st[:, :], in_=agh_o[:, :])
                hg_ps = pst.tile([128, KT], F32, tag="pstf")
                nc.tensor.transpose(hg_ps[:, :], hg_st[:, :], idf[:, :])
                hgc = act.tile([128, KT], BF16, tag=f"hgc{l}")
                nc.vector.tensor_copy(out=hgc[:, :], in_=hg_ps[:, :])
                return hgc

            agh1_i = dram.tile([1, HS], F32, tag="agh1_i")
            agh1_o = dram.tile([KT, 128], F32, tag="agh1_o")
            h1c = lstm_layer(0, xc, wih1s, whh1s, bk1_s, ct[0], agh1_i, agh1_o)
            agh2_i = dram.tile([1, HS], F32, tag="agh2_i")
            agh2_o = dram.tile([KT, 128], F32, tag="agh2_o")
            h2c = lstm_layer(1, h1c, wih2s, whh2s, bk2_s, ct[1], agh2_i, agh2_o)

            # ================= output projection ==========================
            l_ps = psmm.tile([128, 32], F32, tag="l")
            for c in range(4):
                for mm in range(8):
                    m = c * 8 + mm
                    for k in range(KT):
                        nc.tensor.matmul(
                            l_ps[:, m : m + 1],
                            ows[c][:, k * 1024 + mm * 128 : k * 1024 + (mm + 1) * 128],
                            h2c[:, k : k + 1],
                            start=(k == 0),
                            stop=(k == KT - 1),
                        )
            l_sb = act.tile([128, 32], F32, tag="l_sb")
            nc.vector.tensor_tensor(
                out=l_sb[:, :], in0=l_ps[:, :], in1=outbk_s[:, :],
                op=mybir.AluOpType.add,
            )

            # ---- local log-softmax stats ----
            mx = act.tile([128, 1], F32, tag="mx")
            nc.vector.tensor_reduce(
                out=mx[:, :], in_=l_sb[:, :], axis=mybir.AxisListType.X,
                op=mybir.AluOpType.max,
            )
            mxb = act.tile([128, 1], F32, tag="mxb")
            nc.gpsimd.partition_all_reduce(
                mxb[:, :], mx[:, :], 128, bass.bass_isa.ReduceOp.max
            )
            nmx = act.tile([128, 1], F32, tag="nmx")
            nc.vector.tensor_scalar_mul(nmx[:, :], mxb[:, :], -1.0)
            ex = act.tile([128, 32], F32, tag="ex")
            sx = act.tile([128, 1], F32, tag="sx")
            nc.scalar.activation(
                ex[:, :], l_sb[:, :], mybir.ActivationFunctionType.Exp,
                bias=nmx[:, :1], accum_out=sx[:, :],
            )
            sxb = act.tile([128, 1], F32, tag="sxb")
            nc.gpsimd.partition_all_reduce(
                sxb[:, :], sx[:, :], 128, bass.bass_isa.ReduceOp.add
            )

            # ---- AllGather (M_k, S_k) ----
            agst_i = dram.tile([1, 8], F32, tag="agst_i")
            agst_o = dram.tile([8, 8], F32, tag="agst_o")
            stt = act.tile([1, 8], F32, tag="stt")
            nc.vector.memset(stt[:, :], 0.0)
            nc.vector.tensor_copy(out=stt[:, 0:1], in_=mxb[0:1, 0:1])
            nc.vector.tensor_copy(out=stt[:, 1:2], in_=sxb[0:1, 0:1])
            nc.scalar.dma_start(out=agst_i[:, :], in_=stt[:, :])
            nc.gpsimd.collective_compute(
                "AllGather", mybir.AluOpType.bypass, replica_groups=RG,
                ins=[agst_i.opt()], outs=[agst_o.opt()],
            )
            st8 = act.tile([1, 8, 8], F32, tag="st8")
            nc.scalar.dma_start(out=st8[:, :, :], in_=agst_o[:, :])
            mg = act.tile([1, 1], F32, tag="mg")
            nc.vector.tensor_reduce(
                out=mg[:, :], in_=st8[:, :, 0], axis=mybir.AxisListType.X,
                op=mybir.AluOpType.max,
            )
            nmg = act.tile([1, 1], F32, tag="nmg")
            nc.vector.tensor_scalar_mul(nmg[:, :], mg[:, :], -1.0)
            e8 = act.tile([1, 8], F32, tag="e8")
            nc.scalar.activation(
                e8[:, :], st8[:, :, 0], mybir.ActivationFunctionType.Exp,
                bias=nmg[:, :1],
            )
            t8 = act.tile([1, 8], F32, tag="t8")
            nc.vector.tensor_tensor(
                out=t8[:, :], in0=e8[:, :], in1=st8[:, :, 1],
                op=mybir.AluOpType.mult,
            )
            zg = act.tile([1, 1], F32, tag="zg")
            nc.vector.tensor_reduce(
                out=zg[:, :], in_=t8[:, :], axis=mybir.AxisListType.X,
                op=mybir.AluOpType.add,
            )
            lz = act.tile([1, 1], F32, tag="lz")
            nc.scalar.activation(
                lz[:, :], zg[:, :], mybir.ActivationFunctionType.Ln
            )
            shf = act.tile([1, 1], F32, tag="shf")
            nc.vector.tensor_tensor(
                out=shf[:, :], in0=lz[:, :], in1=mg[:, :], op=mybir.AluOpType.add,
            )
            nshf = act.tile([1, 1], F32, tag="nshf")
            nc.vector.tensor_scalar_mul(nshf[:, :], shf[:, :], -1.0)
            nshb = act.tile([128, 1], F32, tag="nshb")
            nc.gpsimd.partition_broadcast(nshb[:, :], nshf[:, :])
            lsm_sb = act.tile([128, 32], F32, tag="lsm_sb")
            nc.vector.tensor_scalar_add(lsm_sb[:, :], l_sb[:, :], nshb[:, :1])
            nc.scalar.dma_start(out=lsm_d[:, :], in_=lsm_sb[:, :])

    nc.compile()
    return nc


def _get_nc():
    if "nc" not in _CACHE:
        _CACHE["nc"] = _build()
    return _CACHE["nc"]


def _prep_in_maps(inputs):
    inp = np.asarray(inputs["input"]).reshape(-1)
    hidden = np.asarray(inputs["hidden"], np.float32)
    cell = np.asarray(inputs["cell"], np.float32)
    enc = np.asarray(inputs["encoder_outputs"], np.float32)
    table = np.asarray(inputs["embed_table"], np.float32)
    attn_W = np.asarray(inputs["attn_W"], np.float32)
    attn_b = np.asarray(inputs["attn_b"], np.float32)
    comb_W = np.asarray(inputs["comb_W"], np.float32)
    comb_b = np.asarray(inputs["comb_b"], np.float32)
    Wih = np.asarray(inputs["Wih"], np.float32)
    Whh = np.asarray(inputs["Whh"], np.float32)
    bih = np.asarray(inputs["bih"], np.float32)
    bhh = np.asarray(inputs["bhh"], np.float32)
    out_W = np.asarray(inputs["out_W"], np.float32)
    out_b = np.asarray(inputs["out_b"], np.float32)

    def kmaj(wt):  # [K, M] -> [128, (K/128)*M] k-tile-major slab
        K, M = wt.shape
        return np.ascontiguousarray(
            wt.reshape(K // 128, 128, M).transpose(1, 0, 2).reshape(128, -1)
        ).astype(np_bf16)

    idx16 = (int(inp[0]) * 16 + np.arange(16, dtype=np.int32)).reshape(16, 1)
    table_bf = np.ascontiguousarray(table.astype(np_bf16).reshape(V * 16, 128))
    hid = [np.ascontiguousarray(hidden[l, 0].reshape(KT, 128)) for l in range(NL)]
    enc_bf = np.ascontiguousarray(enc.astype(np_bf16))
    attnWT = kmaj(np.ascontiguousarray(attn_W.T))
    attnb = np.ascontiguousarray(attn_b.reshape(A, 1))

    # padded out projection
    owp = np.zeros((VP, H), np.float32)
    owp[:V] = out_W
    obp = np.full((VP,), PAD_BIAS, np.float32)
    obp[:V] = out_b

    b2 = bih + bhh  # [NL, 4H]

    common = {
        "idx16": idx16, "table": table_bf, "hid0": hid[0], "hid1": hid[1],
        "enc": enc_bf, "attnWT": attnWT, "attnb": attnb,
    }
    in_maps = []
    for k in range(NC):
        m = dict(common)
        for l in range(NL):
            m[f"cell{l}"] = np.ascontiguousarray(
                cell[l, 0, k * HS : (k + 1) * HS].reshape(2, 128)
            )
        cslice = comb_W[k * HS : (k + 1) * HS]  # [256, 4096]
        m["combWT"] = kmaj(np.ascontiguousarray(cslice.T))
        m["combb"] = np.ascontiguousarray(
            comb_b[k * HS : (k + 1) * HS].reshape(2, 128).T
        )
        for l in range(NL):
            rows = np.concatenate(
                [Wih[l][g * H + k * HS : g * H + (k + 1) * HS] for g in range(4)]
            )  # [1024, 2048]
            m[f"wih{l + 1}"] = kmaj(np.ascontiguousarray(rows.T))
            rows = np.concatenate(
                [Whh[l][g * H + k * HS : g * H + (k + 1) * HS] for g in range(4)]
            )
            m[f"whh{l + 1}"] = kmaj(np.ascontiguousarray(rows.T))
            bk = np.concatenate(
                [b2[l][g * H + k * HS : g * H + (k + 1) * HS] for g in range(4)]
            )  # [1024]
            m[f"bk{l + 1}"] = np.ascontiguousarray(bk.reshape(8, 128).T)
        oslice = np.ascontiguousarray(owp[k * VS : (k + 1) * VS].T)  # [2048, 4096]
        for c in range(4):
            m[f"ow{c}"] = kmaj(np.ascontiguousarray(oslice[:, c * 1024 : (c + 1) * 1024]))
        m["outbk"] = np.ascontiguousarray(
            obp[k * VS : (k + 1) * VS].reshape(32, 128).T
        )
        in_maps.append(m)
    return in_maps


def kernel(**inputs):
    global LAST_EXEC_NS
    import time as _time

    _t0 = _time.time()
    nc = _get_nc()
    print(f"[kernel] graph ready {_time.time() - _t0:.1f}s", file=sys.stderr, flush=True)
    in_maps = _prep_in_maps(inputs)
    print(f"[kernel] host prep done {_time.time() - _t0:.1f}s", file=sys.stderr, flush=True)
    trace = bool(int(os.environ.get("KERNEL_TRACE", "0")))
    res = run_bass_kernel_spmd(
        nc, in_maps, core_ids=list(range(NC)), trace=trace
    )
    print(f"[kernel] run done {_time.time() - _t0:.1f}s", file=sys.stderr, flush=True)
    LAST_EXEC_NS = res.exec_time_ns

    outs = res.results
    lsm = np.concatenate(
        [np.asarray(outs[k]["lsm"]).T.reshape(-1) for k in range(NC)]
    )[:V].reshape(1, V).astype(np.float32)
    h_new = np.stack(
        [
            np.concatenate(
                [np.asarray(outs[k]["hout"][l]).reshape(-1) for k in range(NC)]
            ).reshape(1, H)
            for l in range(NL)
        ]
    )
    c_new = np.stack(
        [
            np.concatenate(
                [np.asarray(outs[k]["cout"][l]).reshape(-1) for k in range(NC)]
            ).reshape(1, H)
            for l in range(NL)
        ]
    )
    attnw = np.asarray(outs[0]["attnw"]).reshape(1, A).astype(np.float32)
    return lsm, h_new, c_new, attnw
